# revision 14
# baseline (speedup 1.0000x reference)
import sys

sys.path.insert(0, "/opt/trn_rl_repo")

import numpy as np
import bass_rust
import concourse.bass as bass
import concourse.mybir as mybir
import concourse.tile as tile
from concourse.bass_utils import run_bass_kernel_spmd

import concourse.dve_ops as dve_ops
from concourse.dve_ops import DveOp
from concourse.dve_spec import Spec, Src0, C0, C1, C2, One, maxx, relu, sq, lower, _has_src1
from concourse.dve_uop import DveOpSpec

B, H, NCORES = 4096, 1024, 8
NB = B // NCORES          # 512 batch rows per core
FB = H // 128             # 8 feature blocks of 128
NPL = 8                   # spline planes per feature
KT_A = 2 * FB * (1 + NPL)   # 144 k-tiles: kan_x + kan_h
KT_B = FB + FB * (1 + NPL)  # 80 k-tiles: hh_w + kan_hz
FP32 = mybir.dt.float32
FP32R = mybir.dt.float32r
AF = mybir.ActivationFunctionType


def _register_dve(name, spec):
    if name in dve_ops._SUB_OPCODE_FOR_NAME:
        return next(op for op in dve_ops.OPS if op.name == name)
    row = dve_ops._CUSTOM_DVE_ROW_BASE + len(dve_ops.OPS)
    assert row < 0x20
    dve_ops._SUB_OPCODE_FOR_NAME[name] = row
    shas = {v: DveOpSpec(name=name, opcode=row, uops=lower(spec, ver=v),
                         rd1_en=_has_src1(spec)).sha(v) for v in ("v3", "v4")}
    op = DveOp(name, spec, subdim=False, uops_sha=shas)
    dve_ops.OPS.append(op)
    dve_ops.CUSTOM_DVE_SPECS[name] = op.spec
    return op


# p_c = relu(2 - |2.5*x - (c - 3.5)|), then plane = p^3 - 4*relu(p-1)^3 = 6*B3(u-c)
_m = Src0 * C2
TENT = _register_dve("KAN_TENT", Spec(
    body=relu(C1 - maxx(_m - C0, C0 - _m)),
    reference=lambda in0, in1, s0, s1, imm2:
        np.maximum(s1 - np.abs(in0.astype(np.float32) * imm2 - s0), 0.0)))
_q = relu(Src0 - One)
CUBE = _register_dve("KAN_CUBE", Spec(
    body=sq(Src0) * Src0 - sq(_q) * _q * C2,
    reference=lambda in0, in1, s0, s1, imm2:
        in0.astype(np.float32) ** 3
        - imm2 * np.maximum(in0.astype(np.float32) - 1.0, 0.0) ** 3))


def _build(reps=1, mode="full"):
    # mode: "full" | "pe" (no weight streaming, one resident w tile)
    #       | "dma" (weight streaming only, no compute)
    nc = bass.Bass(target_bir_lowering=False)
    xs_d = nc.dram_tensor("xs", [128, FB * NB], FP32, kind="ExternalInput")
    zs_d = nc.dram_tensor("zs", [128, FB * NB], FP32R, kind="ExternalInput")
    wa_d = nc.dram_tensor("wa", [KT_A * 128, H], FP32R, kind="ExternalInput")
    wb_d = nc.dram_tensor("wb", [KT_B * 128, H], FP32R, kind="ExternalInput")
    hb_d = nc.dram_tensor("hb", [128, FB], FP32, kind="ExternalInput")
    o_d = nc.dram_tensor("o", [128, FB * NB], FP32, kind="ExternalOutput")
    zo_d = nc.dram_tensor("zo", [128, FB * NB], FP32, kind="ExternalOutput")

    with tile.TileContext(nc) as tc:
        with tc.tile_pool(name="sbuf", bufs=1) as pool, \
             tc.tile_pool(name="wp", bufs=12) as wpool, \
             tc.tile_pool(name="dp", bufs=2) as dpool, \
             tc.tile_pool(name="psum", bufs=1, space="PSUM") as psum:
            xs = pool.tile([128, FB * NB], FP32)
            nc.sync.dma_start(xs[:], xs_d[:])
            zs = pool.tile([128, FB * NB], FP32R)
            nc.sync.dma_start(zs[:], zs_d[:])
            hb = pool.tile([128, FB], FP32)
            nc.sync.dma_start(hb[:], hb_d[:])
            oT = pool.tile([128, FB * NB], FP32R)

            ps = [psum.tile([128, NB], FP32, name=f"ps{mt}") for mt in range(FB)]

            w_res = None
            if mode == "pe":
                w_res = pool.tile([128, H], FP32R)
                nc.sync.dma_start(w_res[:], wa_d[0:128, :])

            def wtile(wd, kt):
                if mode == "pe":
                    return w_res
                w = wpool.tile([128, H], FP32R, name="w")
                nc.sync.dma_start(w[:], wd[kt * 128:(kt + 1) * 128, :])
                return w

            def group(src_fp32, wd, kt, last_kt):
                # one feature block: silu + 8 spline planes -> 9 k-tiles
                sil = dpool.tile([128, NB], FP32R, name="sil")
                nc.scalar.activation(sil[:], src_fp32, AF.Silu)
                tent = dpool.tile([128, NPL * NB], FP32, name="tent")
                for c in range(NPL):
                    nc.vector._custom_dve(TENT, out=tent[:, c * NB:(c + 1) * NB],
                                          in0=src_fp32, s0=float(c) - 3.5, s1=2.0,
                                          imm2=2.5)
                planes = dpool.tile([128, NPL * NB], FP32R, name="planes")
                nc.vector._custom_dve(CUBE, out=planes[:], in0=tent[:], imm2=4.0)
                for j in range(1 + NPL):
                    w = wtile(wd, kt + j)
                    rhs = sil[:] if j == 0 else planes[:, (j - 1) * NB:j * NB]
                    for mt in range(FB):
                        nc.tensor.matmul(ps[mt][:], w[:, mt * 128:(mt + 1) * 128], rhs,
                                         start=(kt + j == 0), stop=(kt + j == last_kt))
                return kt + 1 + NPL

            def dma_only():
                for kt in range(KT_A):
                    w = wpool.tile([128, H], FP32R, name="w")
                    nc.sync.dma_start(w[:], wa_d[kt * 128:(kt + 1) * 128, :])
                for kt in range(KT_B):
                    w = wpool.tile([128, H], FP32R, name="w")
                    nc.sync.dma_start(w[:], wb_d[kt * 128:(kt + 1) * 128, :])

            for _rep in range(reps):
                if mode == "dma":
                    dma_only()
                    continue
                # phase A: s = kan_x(x) + kan_h(z); o = tanh(s)
                kt = 0
                for src_tile, is_r in ((xs, False), (zs, True)):
                    for fb in range(FB):
                        sl = src_tile[:, fb * NB:(fb + 1) * NB]
                        kt = group(sl.bitcast(FP32) if is_r else sl, wa_d, kt, KT_A - 1)
                for mt in range(FB):
                    nc.scalar.activation(oT[:, mt * NB:(mt + 1) * NB], ps[mt][:], AF.Tanh)
                    nc.sync.dma_start(o_d[:, mt * NB:(mt + 1) * NB],
                                      oT[:, mt * NB:(mt + 1) * NB].bitcast(FP32))

                # phase B: z_out = z @ hh_w.T + hh_b + kan_hz(o)
                kt = 0
                for fb in range(FB):
                    w = wtile(wb_d, kt)
                    rhs = zs[:, fb * NB:(fb + 1) * NB]
                    for mt in range(FB):
                        nc.tensor.matmul(ps[mt][:], w[:, mt * 128:(mt + 1) * 128], rhs,
                                         start=(kt == 0), stop=False)
                    kt += 1
                for fb in range(FB):
                    kt = group(oT[:, fb * NB:(fb + 1) * NB].bitcast(FP32), wb_d,
                               kt, KT_B - 1)
                for mt in range(FB):
                    zst = dpool.tile([128, NB], FP32, name="zst")
                    nc.scalar.activation(zst[:], ps[mt][:], AF.Identity,
                                         bias=hb[:, mt:mt + 1], scale=1.0)
                    nc.sync.dma_start(zo_d[:, mt * NB:(mt + 1) * NB], zst[:])

    bass_rust.generate_event_semaphores(nc)
    mybir.codegen_inst_isa_subclasses(nc)
    return nc


_NCS = {}


def _get_nc(reps=1, mode="full"):
    key = (reps, mode)
    if key not in _NCS:
        _NCS[key] = _build(reps, mode)
    return _NCS[key]


def _to_dev(a):  # [NB, H] -> [128, FB*NB], block fb holds features fb*128..+128
    return np.ascontiguousarray(
        a.reshape(NB, FB, 128).transpose(2, 1, 0).reshape(128, FB * NB))


def _from_dev(a):  # [128, FB*NB] -> [NB, H]
    return a.reshape(128, FB, NB).transpose(2, 1, 0).reshape(NB, H)


def _pack_kan(wa, kt, bw, sw, sc):
    w2 = (np.asarray(sw, np.float32) * np.asarray(sc, np.float32)[:, :, None]) / 6.0
    bw = np.asarray(bw, np.float32)
    for fb in range(FB):
        wa[kt * 128:(kt + 1) * 128] = bw[:, fb * 128:(fb + 1) * 128].T
        kt += 1
        for c in range(NPL):
            wa[kt * 128:(kt + 1) * 128] = w2[:, fb * 128:(fb + 1) * 128, c].T
            kt += 1
    return kt


def _make_in_maps(inputs):
    x = np.ascontiguousarray(np.asarray(inputs["x_t"], np.float32))
    z = np.ascontiguousarray(np.asarray(inputs["z_prev"], np.float32))

    wa = np.empty((KT_A * 128, H), np.float32)
    kt = _pack_kan(wa, 0, inputs["wx_base"], inputs["wx_spline"], inputs["wx_scaler"])
    kt = _pack_kan(wa, kt, inputs["wh_base"], inputs["wh_spline"], inputs["wh_scaler"])
    assert kt == KT_A

    wb = np.empty((KT_B * 128, H), np.float32)
    wb[:H] = np.asarray(inputs["hh_w"], np.float32).T
    kt = _pack_kan(wb, FB, inputs["hz_base"], inputs["hz_spline"], inputs["hz_scaler"])
    assert kt == KT_B

    hb = np.ascontiguousarray(
        np.asarray(inputs["hh_b"], np.float32).reshape(FB, 128).T)

    return [{"xs": _to_dev(x[d * NB:(d + 1) * NB]),
             "zs": _to_dev(z[d * NB:(d + 1) * NB]),
             "wa": wa, "wb": wb, "hb": hb} for d in range(NCORES)]


def _run(inputs, trace=False):
    nc = _get_nc()
    in_maps = _make_in_maps(inputs)
    res = run_bass_kernel_spmd(nc, in_maps, list(range(NCORES)), trace=trace)
    o = np.empty((B, H), np.float32)
    zt = np.empty((B, H), np.float32)
    for d in range(NCORES):
        o[d * NB:(d + 1) * NB] = _from_dev(res.results[d]["o"])
        zt[d * NB:(d + 1) * NB] = _from_dev(res.results[d]["zo"])
    return (o, zt), res


def kernel(**inputs):
    return _run(inputs, trace=False)[0]
